# revision 19
# baseline (speedup 1.0000x reference)
"""Channel attention kernel for Trainium2, 8-core data parallel.

Computes, per batch b:
    X   = x[b].reshape(C, H*W)            # (512, 2304)
    G   = X @ X.T                         # (512, 512) Gram
    A   = softmax(G, axis=1)
    agg = A @ X                           # (512, 2304)
    out[b] = x[b] + scale * agg

Sharding: pure data parallel over the batch dim n=64 -> 8 batches per core.

Per-core pipeline (all in fp32, matmuls in float32r fast mode):
  1. DMA x[b] into 4 SBUF tiles X[cb] = [128, 2304]
  2. PE-transpose X -> XT (18 tiles [128d, 512c]) via identity matmul
  3. mm1: G[mb] (PSUM, [128,512]) += XT[kb][:,mb]^T @ XT[kb]   (kb = 0..17)
  4. softmax: row max (DVE, negated) -> exp w/ bias + row-sum (ACT, fused
     accum) -> reciprocal; normalization folded into the final residual
  5. PE-transpose E -> ET (4 tiles [128k, 512c])
  6. mm2: Y (PSUM) += ET[kb][:,mb]^T @ X[kb][:, chunk]
  7. out = (Y * (scale/S[row])) + X  in one DVE scalar_tensor_tensor, DMA out
"""

import numpy as np
from contextlib import ExitStack

import concourse.bass as bass
import concourse.bacc as bacc
import concourse.tile as tile
from concourse import mybir
from concourse.masks import make_identity
from concourse.bass_utils import run_bass_kernel_spmd

N_CORES = 8
N, C, H, W = 64, 512, 48, 48
HW = H * W                    # 2304
B = N // N_CORES              # 8 batches per core
P = 128
NCB = C // P                  # 4 c-blocks
NDB = HW // P                 # 18 d-blocks
F32 = mybir.dt.float32
F32R = mybir.dt.float32r

# d-chunks for mm2 / residual / store: 4 x 512 + 1 x 256
CHUNKS = [(i * 512, min(512, HW - i * 512)) for i in range((HW + 511) // 512)]

AX = mybir.AxisListType.X
MULT = mybir.AluOpType.mult
ADD = mybir.AluOpType.add
EXP = mybir.ActivationFunctionType.Exp
COPY = mybir.ActivationFunctionType.Copy


def _r(ap):
    """View an fp32 AP as float32r for full-rate PE streaming."""
    return ap.bitcast(F32R)


def _build(use_f32r=True):
    nc = bacc.Bacc()
    x_d = nc.dram_tensor("x", [B, C, HW], F32, kind="ExternalInput")
    s_d = nc.dram_tensor("scale", [1], F32, kind="ExternalInput")
    o_d = nc.dram_tensor("out", [B, C, HW], F32, kind="ExternalOutput")

    MMDT = F32R if use_f32r else F32

    with tile.TileContext(nc) as tc:
        with ExitStack() as ctx:
            singles = ctx.enter_context(tc.tile_pool(name="singles", bufs=1))
            xpool = ctx.enter_context(tc.tile_pool(name="xp", bufs=8))
            xtpool = ctx.enter_context(tc.tile_pool(name="xtp", bufs=19))
            epool = ctx.enter_context(tc.tile_pool(name="ep", bufs=6))
            etpool = ctx.enter_context(tc.tile_pool(name="etp", bufs=6))
            opool = ctx.enter_context(tc.tile_pool(name="op", bufs=6))
            stats = ctx.enter_context(tc.tile_pool(name="st", bufs=24))
            tpsum = ctx.enter_context(
                tc.tile_pool(name="tps", bufs=4, space="PSUM"))
            gpsum = ctx.enter_context(
                tc.tile_pool(name="gps", bufs=2, space="PSUM"))
            ypsum = ctx.enter_context(
                tc.tile_pool(name="yps", bufs=2, space="PSUM"))

            identity = singles.tile([P, P], F32)
            make_identity(nc, identity[:])
            scale_sb = singles.tile([P, 1], F32)
            nc.sync.dma_start(out=scale_sb[:], in_=s_d.broadcast_to([P, 1]))

            # Dummy transpose so the PE observes the gpsimd-produced
            # identity once here; real matmuls then never need that wait.
            # (Matmul instructions have a single sync-wait slot in walrus
            # codegen, so each must depend on at most one fresh semaphore.)
            # It writes into the G pool: the next user is a PE matmul, and
            # same-engine WAW ordering needs no semaphore.
            warm = gpsum.tile([P, P], F32, tag="g")
            nc.tensor.transpose(warm[:], identity[:], identity[:])

            prev_last_mm1 = None
            for b in range(B):
                # ---- load X (natural layout, 4 tiles of [128, 2304]) ----
                # Tiles are declared f32r so the PE can consume them at
                # full rate; the DMA moves raw fp32 bits (bitcast), and the
                # residual later bitcasts back to f32 for an exact read.
                xs = []
                for cb in range(NCB):
                    xt = xpool.tile([P, HW], MMDT, tag="x")
                    nc.sync.dma_start(
                        out=xt[:],
                        in_=x_d[b, cb * P:(cb + 1) * P, :].bitcast(MMDT))
                    xs.append(xt)

                # ---- transpose X -> XT (PE), evacuate on DVE ----
                xT = [xtpool.tile([P, C], MMDT, tag="xt", name=f"xT{kb}") for kb in range(NDB)]
                for kb in range(NDB):
                    for cb in range(NCB):
                        ps = tpsum.tile([P, P], F32, tag="tps")
                        tr = nc.tensor.transpose(
                            ps[:],
                            xs[cb][:, kb * P:(kb + 1) * P].bitcast(F32),
                            identity[:])
                        if kb == 0 and prev_last_mm1 is not None:
                            # Keep the DMA-waiting transposes of batch b
                            # behind batch b-1's mm1 tail so their PSUM-slot
                            # releases are already observed ticks and each
                            # carries only its single DMA wait.
                            tile.add_dep_helper(
                                tr.ins, prev_last_mm1.ins, sync=False,
                                reason="1-wait-slot: order after prev mm1")
                        nc.vector.tensor_copy(
                            out=xT[kb][:, cb * P:(cb + 1) * P], in_=ps[:])

                # ---- mm1 (one G bank at a time) + softmax + E transpose --
                # G's two PSUM readers are both DVE (rmax + evacuate copy)
                # so the bank release is a single semaphore; exp reads the
                # SBUF copy on ACT; E-transpose results evacuate on ACT so
                # the eT chain also rides one semaphore.
                eT = [etpool.tile([P, C], MMDT, tag="et", name=f"eT{kb}") for kb in range(NCB)]
                alphas = []
                for mb in range(NCB):
                    G = gpsum.tile([P, C], F32, tag="g", name=f"G{mb}")
                    for kb in range(NDB):
                        mm = nc.tensor.matmul(
                            G[:],
                            xT[kb][:, mb * P:(mb + 1) * P],
                            xT[kb][:],
                            start=(kb == 0), stop=(kb == NDB - 1))
                        prev_last_mm1 = mm
                    neg_m = stats.tile([P, 1], F32, tag="negm")
                    nc.vector.reduce_max(
                        out=neg_m[:], in_=G[:], axis=AX, negate=True)
                    g_sb = epool.tile([P, C], F32, tag="gsb")
                    nc.vector.tensor_copy(out=g_sb[:], in_=G[:])
                    e = epool.tile([P, C], F32, tag="e")
                    s = stats.tile([P, 1], F32, tag="s")
                    nc.scalar.activation(
                        out=e[:], in_=g_sb[:], func=EXP,
                        bias=neg_m[:], scale=1.0, accum_out=s[:])
                    rs = stats.tile([P, 1], F32, tag="rs")
                    nc.vector.reciprocal(out=rs[:], in_=s[:])
                    alpha = stats.tile([P, 1], F32, tag="al")
                    nc.vector.tensor_mul(alpha[:], rs[:], scale_sb[:])
                    alphas.append(alpha)
                    # E-transposes share the Y psum pool; their ACT
                    # evacuations keep the whole eT chain on one semaphore.
                    for kb in range(NCB):
                        ps = ypsum.tile([P, P], F32, tag="y", name="eps")
                        nc.tensor.transpose(
                            ps[:], e[:, kb * P:(kb + 1) * P], identity[:])
                        nc.scalar.copy(
                            out=eT[kb][:, mb * P:(mb + 1) * P], in_=ps[:])

                # ---- mm2 + fused residual + store ----
                # The last two Y evacuations of the batch go through ACT so
                # both Y-pool slots are ACT-released when the next batch's
                # first E-transpose (which already waits on ACT for its
                # exp input) grabs them.
                for mb in range(NCB):
                    for ci, (c0, csz) in enumerate(CHUNKS):
                        y = ypsum.tile([P, 512], F32, tag="y")
                        for kb in range(NCB):
                            nc.tensor.matmul(
                                y[:, :csz],
                                eT[kb][:, mb * P:(mb + 1) * P],
                                xs[kb][:, c0:c0 + csz],
                                start=(kb == 0), stop=(kb == NCB - 1))
                        o = opool.tile([P, 512], F32, tag="o")
                        if mb == NCB - 1 and ci >= len(CHUNKS) - 2:
                            ysb = opool.tile([P, 512], F32, tag="ysb")
                            nc.scalar.activation(
                                out=ysb[:, :csz], in_=y[:, :csz],
                                func=COPY, bias=0.0, scale=alphas[mb][:])
                            nc.vector.tensor_add(
                                o[:, :csz], ysb[:, :csz],
                                xs[mb][:, c0:c0 + csz].bitcast(F32))
                        else:
                            nc.vector.scalar_tensor_tensor(
                                out=o[:, :csz], in0=y[:, :csz],
                                scalar=alphas[mb][:],
                                in1=xs[mb][:, c0:c0 + csz].bitcast(F32),
                                op0=MULT, op1=ADD)
                        nc.sync.dma_start(
                            out=o_d[b, mb * P:(mb + 1) * P, c0:c0 + csz],
                            in_=o[:, :csz])
    nc.finalize()
    return nc


def _ensure_ntff_hook():
    """Install the axon NTFF profiling hook if the image's antenv lacks it.

    Only needed for trace=True runs (local perf iteration); the grading
    path never calls this.
    """
    import sys
    import types
    try:
        from antenv import axon_hooks  # noqa: F401
        return
    except ImportError:
        pass
    mod = types.ModuleType("antenv.axon_hooks")
    _h = {"hook": None}
    mod.set_axon_ntff_profile_hook = lambda h: _h.__setitem__("hook", h)
    mod.get_axon_ntff_profile_hook = lambda: _h["hook"]
    sys.modules["antenv.axon_hooks"] = mod
    import antenv
    antenv.axon_hooks = mod
    try:
        from trn_agent_boot.trn_boot import _ntff_profile_via_ctypes
        hook = _ntff_profile_via_ctypes("/opt/axon/libaxon_pjrt.so")
        if hook is not None:
            mod.set_axon_ntff_profile_hook(hook)
    except Exception:
        pass


_NC_CACHE = {}


def _get_nc(use_f32r=True):
    if use_f32r not in _NC_CACHE:
        _NC_CACHE[use_f32r] = _build(use_f32r)
    return _NC_CACHE[use_f32r]


def kernel(x, scale, trace=False, use_f32r=True):
    x = np.ascontiguousarray(x, dtype=np.float32)
    scale = np.ascontiguousarray(scale, dtype=np.float32)
    if trace:
        _ensure_ntff_hook()
    nc = _get_nc(use_f32r)
    xr = x.reshape(N, C, HW)
    in_maps = [
        {"x": xr[i * B:(i + 1) * B], "scale": scale}
        for i in range(N_CORES)
    ]
    res = run_bass_kernel_spmd(
        nc, in_maps, core_ids=list(range(N_CORES)), trace=trace)
    out = np.concatenate([r["out"] for r in res.results], axis=0)
    out = out.reshape(N, C, H, W)
    if trace:
        kernel.last_exec_time_ns = res.exec_time_ns
        kernel.last_results = res
    return out
